# revision 40
# baseline (speedup 1.0000x reference)
"""MBConv (4D spatial, 16^4) on 8 TRN2 NeuronCores.

Sharding: spatial-parallel over the first spatial dim X (16 planes ->
2 owned planes per core + 1 halo plane each side, shipped from host).

Math (all on device except weight-only constant folding on host):
  GN0+conv1+GN1 folded: A' = (W1 * g0_w) . x computed once; the two
  global groupnorms reduce to 6 scalars in ONE AllReduce:
    [Sum(A'), Sum(A'^2), Sum(u*SA), Sum(v*SA), Sum(x), Sum(x^2)]
  with u = W1.g0_b, v = W1.g0_w (host constants); then
  h1 = gelu(alpha1 * A' + beta1) per hidden channel.
  conv2 = 81 accumulating PE matmuls per PSUM bank over a zero-padded
  [128ch, 4planes, 18,18,18] SBUF tile (float32r fast-fp32 mode).
  GN2 -> AllReduce(2 scalars); gelu fused with SE partial-mean accum.
  SE mean -> AllReduce(128); SE MLP on-device; scale folded into w3.
  conv3; GN3 -> AllReduce(2 scalars); affine; DMA out.
"""

import sys
sys.path.insert(0, '/opt/trn_rl_repo')

import numpy as np
import ml_dtypes

import concourse.bass as bass
import concourse.bacc as bacc
import concourse.tile as tile
import concourse.mybir as mybir
from concourse.bass_utils import run_bass_kernel_spmd

F32 = mybir.dt.float32
F32R = mybir.dt.float32r
BF16 = mybir.dt.bfloat16
AF = mybir.ActivationFunctionType

N_CORES = 8
S = 16
CIN = 32
HID = 128
EPS = 1e-5
PLANE = S * S * S            # 4096 positions per x-plane
PPAD = 18 * 18 * 18          # padded plane (z/y/w pad 1)
NPL = 4                      # stored planes per core (2 owned + 2 halo)
POS = 2 * PLANE              # owned positions per core
P_SP = S ** 4                # 65536 global spatial positions
NX = CIN * P_SP
N1 = HID * P_SP
N3 = CIN * P_SP

_cache = {}


def _col(t, i):
    return t[:, i:i + 1]


def build_program(trace_scopes=False):
    nc = bacc.Bacc("TRN2", target_bir_lowering=False, debug=False,
                   enable_asserts=False, num_devices=N_CORES)

    xs_d = nc.dram_tensor("xs", [128, PLANE], F32R, kind="ExternalInput").ap()
    w1_d = nc.dram_tensor("w1rep", [128, 128], F32R, kind="ExternalInput").ap()
    w2_d = nc.dram_tensor("w2t", [128, 81 * 128], BF16, kind="ExternalInput").ap()
    pp_d = nc.dram_tensor("params", [128, 192], F32, kind="ExternalInput").ap()
    out_d = nc.dram_tensor("out", [CIN, POS], F32, kind="ExternalOutput").ap()

    with tile.TileContext(nc) as tc:
        with tc.tile_pool(name="big", bufs=1) as big, \
             tc.tile_pool(name="small", bufs=1) as small, \
             tc.tile_pool(name="scr", bufs=24) as scr, \
             tc.tile_pool(name="ps", bufs=8, space="PSUM") as ps, \
             tc.tile_pool(name="dram", bufs=1, space="DRAM") as dram:

            def stile(shape, name, pool=None):
                return (pool or small).tile(shape, F32, name=name)

            def sc(name):
                return scr.tile([128, 1], F32, tag="scr", name=name)

            # ---- persistent SBUF tensors ----
            x_sb = big.tile([128, PLANE], F32R, name="x_sb")
            w1_sb = big.tile([128, 128], F32R, name="w1_sb")
            w2_sb = big.tile([128, 81 * 128], BF16, name="w2_sb")
            pp = big.tile([128, 192], F32, name="pp")
            h1 = big.tile([128, NPL * PPAD], BF16, name="h1", tag="bigslot")
            h2 = big.tile([128, 2 * PLANE], F32R, name="h2")

            nc.sync.dma_start(out=x_sb, in_=xs_d)
            nc.sync.dma_start(out=pp, in_=pp_d)
            nc.sync.dma_start(out=w1_sb, in_=w1_d)
            nc.sync.dma_start(out=w2_sb, in_=w2_d)

            h1f5 = h1.rearrange("p (j y z w) -> p j y z w", j=NPL, y=18, z=18, w=18)
            h1pl = h1.rearrange("p (j r) -> p j r", j=NPL, r=PPAD)
            # zero h1 (padding must be 0)
            for j in range(NPL):
                eng = nc.vector if j % 2 == 0 else nc.gpsimd
                eng.memset(h1pl[:, j, :], 0.0)

            def interior(j):
                return h1f5[:, j, 1:17, 1:17, 1:17]

            def interior_chunk(j, n):  # output y-pair chunk [128,(2,16,16)]
                return h1f5[:, j, 1 + 2 * n:3 + 2 * n, 1:17, 1:17]

            eps_t = stile([128, 1], "eps_t")
            nc.vector.memset(eps_t, EPS)
            ones = stile([128, 1], "ones")
            nc.vector.memset(ones, 1.0)

            # ---- conv1: A' = (W1*g0w) . x  on all 4 planes ----
            # Shard partition packing puts OWNED planes on partitions 0:64
            # (stored order [owned0, owned1, haloL, haloR]); LOC maps stored
            # plane index -> local x position in the padded h1 buffer.
            # A'-stats (owned planes only) taken from the contiguous PSUM
            # tiles before eviction (bn_stats reduces innermost dim only).
            LOC = (1, 2, 0, 3)
            sta = stile([128, 16, 6], "sta")
            for sj in range(NPL):
                lj = LOC[sj]
                for n in range(8):
                    pt = ps.tile([128, 512], F32, tag="ps", name=f"c1_{sj}_{n}")
                    nc.tensor.matmul(
                        pt,
                        w1_sb[32 * sj:32 * sj + 32, :],
                        x_sb[32 * sj:32 * sj + 32, bass.ts(n, 512)],
                        start=True, stop=True, tile_position=(32 * sj, 0))
                    nc.scalar.copy(
                        out=interior_chunk(lj, n),
                        in_=pt.rearrange("p (y z w) -> p y z w", y=2, z=16, w=16))
                    if sj < 2:
                        nc.vector.bn_stats(out=sta[:, sj * 8 + n, :], in_=pt)

            # ---- stats for folded GN0+GN1 (owned data only) ----
            stx = stile([128, 8, 6], "stx")
            x_f32 = x_sb.bitcast(F32)
            for c in range(8):
                nc.vector.bn_stats(out=stx[0:64, c, :],
                                   in_=x_f32[0:64, bass.ts(c, 512)])
            mvx = stile([128, 2], "mvx")
            nc.vector.bn_aggr(out=mvx[0:64, :], in_=stx[0:64])

            mva = stile([128, 2], "mva")
            nc.vector.bn_aggr(out=mva, in_=sta)

            pk = stile([128, 6], "pk")
            nc.vector.memset(pk, 0.0)
            # col0: SA_o = mean*POS ; col1: SAA_o = (var+mean^2)*POS
            nc.scalar.mul(out=_col(pk, 0), in_=_col(mva, 0), mul=float(POS))
            t_a = sc("t_a")
            nc.vector.tensor_mul(t_a, _col(mva, 0), _col(mva, 0))
            nc.vector.tensor_add(t_a, t_a, _col(mva, 1))
            nc.scalar.mul(out=_col(pk, 1), in_=t_a, mul=float(POS))
            nc.vector.tensor_mul(_col(pk, 2), _col(pp, 0), _col(pk, 0))   # u*SA
            nc.vector.tensor_mul(_col(pk, 3), _col(pp, 1), _col(pk, 0))   # v*SA
            # x stats on owned planes (partitions 0:64, 4096 positions each)
            nc.scalar.mul(out=pk[0:64, 4:5], in_=mvx[0:64, 0:1], mul=float(PLANE))
            t_b = sc("t_b")
            nc.vector.tensor_mul(t_b[0:64], mvx[0:64, 0:1], mvx[0:64, 0:1])
            nc.vector.tensor_add(t_b[0:64], t_b[0:64], mvx[0:64, 1:2])
            nc.scalar.mul(out=pk[0:64, 5:6], in_=t_b[0:64], mul=float(PLANE))

            ps_s1 = ps.tile([1, 6], F32, tag="ps", name="ps_s1")
            nc.tensor.matmul(ps_s1, ones, pk, start=True, stop=True)

            row1 = stile([1, 8], "row1")
            nc.vector.memset(row1, 0.0)
            nc.vector.tensor_copy(out=row1[0:1, 0:6], in_=ps_s1)
            d1i = dram.tile([8], F32, name="d1i")
            d1o = dram.tile([8], F32, name="d1o")
            nc.sync.dma_start(out=d1i, in_=row1)
            nc.gpsimd.collective_compute(
                "AllReduce", mybir.AluOpType.add,
                replica_groups=[list(range(N_CORES))],
                ins=[d1i.opt()], outs=[d1o.opt()])
            g1 = stile([128, 8], "g1")
            nc.sync.dma_start(out=g1, in_=bass.AP(
                tensor=d1o.tensor, offset=d1o.offset, ap=[[0, 128]] + list(d1o.ap)))

            # ---- scalar chain (replicated on 128 partitions) ----
            def gn_mu_r(g, i_sum, i_ss, nval, tag):
                mu = stile([128, 1], f"mu_{tag}")
                nc.scalar.mul(out=mu, in_=_col(g, i_sum), mul=1.0 / nval)
                ex2 = sc(f"ex2_{tag}")
                nc.scalar.mul(out=ex2, in_=_col(g, i_ss), mul=1.0 / nval)
                var = sc(f"var_{tag}")
                nc.vector.tensor_mul(var, mu, mu)
                nc.vector.tensor_sub(var, ex2, var)
                std = sc(f"std_{tag}")
                nc.scalar.activation(out=std, in_=var, func=AF.Sqrt, bias=eps_t)
                r = stile([128, 1], f"r_{tag}")
                nc.vector.reciprocal(r, std)
                return mu, r

            # g1 cols: 0 SumSA, 1 SAA, 2 SumU.SA, 3 SumV.SA, 4 Sx, 5 Sxx
            mu0, r0 = gn_mu_r(g1, 4, 5, NX, "0")
            q = stile([128, 1], "q")
            nc.vector.tensor_mul(q, mu0, r0)
            scsa = sc("scsa")                       # Sum(c*SA) = col2 - q*col3
            nc.vector.tensor_mul(scsa, q, _col(g1, 3))
            nc.vector.tensor_sub(scsa, _col(g1, 2), scsa)
            s_c = sc("s_c")                         # Sum(c) = Su - q*Sv
            nc.vector.tensor_mul(s_c, q, _col(pp, 11))
            nc.vector.tensor_sub(s_c, _col(pp, 10), s_c)
            scc = sc("scc")                         # Sum(c^2)
            t_c = sc("t_c")
            nc.vector.tensor_mul(t_c, q, _col(pp, 13))
            nc.scalar.mul(out=t_c, in_=t_c, mul=2.0)
            nc.vector.tensor_sub(scc, _col(pp, 12), t_c)
            nc.vector.tensor_mul(t_c, q, q)
            nc.vector.tensor_mul(t_c, t_c, _col(pp, 14))
            nc.vector.tensor_add(scc, scc, t_c)
            # mu1
            mu1 = stile([128, 1], "mu1")
            nc.vector.tensor_mul(mu1, r0, _col(g1, 0))
            t_d = sc("t_d")
            nc.scalar.mul(out=t_d, in_=s_c, mul=float(P_SP))
            nc.vector.tensor_add(mu1, mu1, t_d)
            nc.scalar.mul(out=mu1, in_=mu1, mul=1.0 / N1)
            # var1 = (r0^2*SAA + 2 r0 scsa + P*scc)/N1 - mu1^2
            v1 = sc("v1")
            nc.vector.tensor_mul(v1, r0, r0)
            nc.vector.tensor_mul(v1, v1, _col(g1, 1))
            t_e = sc("t_e")
            nc.vector.tensor_mul(t_e, r0, scsa)
            nc.scalar.mul(out=t_e, in_=t_e, mul=2.0)
            nc.vector.tensor_add(v1, v1, t_e)
            nc.scalar.mul(out=t_e, in_=scc, mul=float(P_SP))
            nc.vector.tensor_add(v1, v1, t_e)
            nc.scalar.mul(out=v1, in_=v1, mul=1.0 / N1)
            nc.vector.tensor_mul(t_e, mu1, mu1)
            nc.vector.tensor_sub(v1, v1, t_e)
            std1 = sc("std1")
            nc.scalar.activation(out=std1, in_=v1, func=AF.Sqrt, bias=eps_t)
            r1 = stile([128, 1], "r1")
            nc.vector.reciprocal(r1, std1)
            al1 = stile([128, 1], "al1")
            nc.vector.tensor_mul(al1, r0, r1)
            nc.vector.tensor_mul(al1, al1, _col(pp, 2))
            be1 = stile([128, 1], "be1")
            nc.vector.tensor_mul(be1, q, _col(pp, 1))        # q*v
            nc.vector.tensor_sub(be1, _col(pp, 0), be1)      # c = u - q*v
            nc.vector.tensor_sub(be1, be1, mu1)              # c - mu1
            nc.vector.tensor_mul(be1, be1, r1)
            nc.vector.tensor_mul(be1, be1, _col(pp, 2))
            nc.vector.tensor_add(be1, be1, _col(pp, 3))

            # ---- apply gelu(alpha1*A' + beta1) in place; mask edge halos ----
            for j in range(NPL):
                nc.scalar.activation(out=interior(j), in_=interior(j),
                                     func=AF.Gelu, bias=be1, scale=al1)
            nc.vector.tensor_scalar_mul(out=interior(0), in0=interior(0),
                                        scalar1=_col(pp, 8))
            nc.vector.tensor_scalar_mul(out=interior(NPL - 1), in0=interior(NPL - 1),
                                        scalar1=_col(pp, 9))

            # ---- conv2: 3^4, 81 taps, accumulate in PSUM ----
            h1r5 = h1f5
            w2r = w2_sb
            sth = stile([128, 16, 6], "sth")
            for j in range(2):
                for b in range(8):
                    # all 81 taps accumulate into ONE psum bank back-to-back
                    # (bank-contiguous: avoids PSUM queue cycling + keeps the
                    # LDWEIGHTS/MATMUL pipeline warm); eviction of bank b
                    # overlaps bank b+1's accumulation
                    pt = ps.tile([128, 512], F32, tag="ps", name=f"c2_{j}_{b}")
                    t = 0
                    for dx in range(3):
                        for dy in range(3):
                            for dz in range(3):
                                for dw in range(3):
                                    mov = h1r5[:, j + dx,
                                               2 * b + dy:2 * b + dy + 2,
                                               dz:dz + 16, dw:dw + 16]
                                    nc.tensor.matmul(pt, w2r[:, bass.ts(t, 128)],
                                                     mov,
                                                     start=(t == 0), stop=(t == 80))
                                    t += 1
                    blk = bass.ts(j * 8 + b, 512)
                    nc.scalar.copy(out=h2[:, blk], in_=pt)
                    nc.vector.bn_stats(out=sth[:, j * 8 + b, :],
                                       in_=h2.bitcast(F32)[:, blk])

            mvh = stile([128, 2], "mvh")
            nc.vector.bn_aggr(out=mvh, in_=sth)
            pk2 = stile([128, 2], "pk2")
            nc.scalar.mul(out=_col(pk2, 0), in_=_col(mvh, 0), mul=float(POS))
            t_f = sc("t_f")
            nc.vector.tensor_mul(t_f, _col(mvh, 0), _col(mvh, 0))
            nc.vector.tensor_add(t_f, t_f, _col(mvh, 1))
            nc.scalar.mul(out=_col(pk2, 1), in_=t_f, mul=float(POS))
            ps_s2 = ps.tile([1, 2], F32, tag="ps", name="ps_s2")
            nc.tensor.matmul(ps_s2, ones, pk2, start=True, stop=True)
            row2 = stile([1, 8], "row2")
            nc.vector.memset(row2, 0.0)
            nc.vector.tensor_copy(out=row2[0:1, 0:2], in_=ps_s2)
            d2i = dram.tile([8], F32, name="d2i")
            d2o = dram.tile([8], F32, name="d2o")
            nc.sync.dma_start(out=d2i, in_=row2)
            nc.gpsimd.collective_compute(
                "AllReduce", mybir.AluOpType.add,
                replica_groups=[list(range(N_CORES))],
                ins=[d2i.opt()], outs=[d2o.opt()])
            g2 = stile([128, 8], "g2")
            nc.sync.dma_start(out=g2, in_=bass.AP(
                tensor=d2o.tensor, offset=d2o.offset, ap=[[0, 128]] + list(d2o.ap)))

            mu2, r2 = gn_mu_r(g2, 0, 1, N1, "2")
            al2 = stile([128, 1], "al2")
            nc.vector.tensor_mul(al2, r2, _col(pp, 4))
            be2 = stile([128, 1], "be2")
            nc.vector.tensor_mul(be2, mu2, al2)
            nc.vector.tensor_sub(be2, _col(pp, 5), be2)

            # ---- gelu(GN2) in place + SE partial sums via accum_out ----
            mcols = stile([128, 16], "mcols")
            h2f = h2.bitcast(F32)
            for n in range(16):
                nc.scalar.activation(out=h2[:, bass.ts(n, 512)],
                                     in_=h2f[:, bass.ts(n, 512)],
                                     func=AF.Gelu, bias=be2, scale=al2,
                                     accum_out=mcols[:, n:n + 1])
            m_col = stile([128, 1], "m_col")
            nc.vector.reduce_sum(out=m_col, in_=mcols, axis=mybir.AxisListType.X)
            d3i = dram.tile([128], F32, name="d3i")
            d3o = dram.tile([128], F32, name="d3o")
            nc.sync.dma_start(out=d3i, in_=m_col)
            nc.gpsimd.collective_compute(
                "AllReduce", mybir.AluOpType.add,
                replica_groups=[list(range(N_CORES))],
                ins=[d3i.opt()], outs=[d3o.opt()])
            m_sb = stile([128, 1], "m_sb")
            nc.sync.dma_start(out=m_sb, in_=d3o)

            # ---- SE MLP (tiny, replicated on every core) ----
            m_mean = stile([128, 1], "m_mean")
            nc.scalar.mul(out=m_mean, in_=m_sb, mul=1.0 / P_SP)
            ps_se1 = ps.tile([8, 1], F32, tag="ps", name="ps_se1")
            nc.tensor.matmul(ps_se1, pp[:, 16:24], m_mean, start=True, stop=True)
            y1g = stile([8, 1], "y1g")
            nc.scalar.activation(out=y1g, in_=ps_se1, func=AF.Gelu)
            ps_se2 = ps.tile([128, 1], F32, tag="ps", name="ps_se2")
            nc.tensor.matmul(ps_se2, pp[0:8, 56:184], y1g, start=True, stop=True)
            s_sb = stile([128, 1], "s_sb")
            nc.scalar.activation(out=s_sb, in_=ps_se2, func=AF.Sigmoid)
            w3s = small.tile([128, 32], F32R, name="w3s")
            nc.vector.tensor_scalar_mul(out=w3s, in0=pp[:, 24:56], scalar1=s_sb)

            # ---- conv3 (+ stats), y3 shares the h1 slot ----
            y3 = big.tile([CIN, POS], F32, name="y3", tag="bigslot")
            st3 = stile([32, 16, 6], "st3")
            for n in range(16):
                pt3 = ps.tile([32, 512], F32, tag="ps", name=f"c3_{n}")
                nc.tensor.matmul(pt3, w3s, h2[:, bass.ts(n, 512)],
                                 start=True, stop=True)
                nc.scalar.copy(out=y3[:, bass.ts(n, 512)], in_=pt3)
                nc.vector.bn_stats(out=st3[:, n, :], in_=pt3)
            mv3 = stile([32, 2], "mv3")
            nc.vector.bn_aggr(out=mv3, in_=st3)
            pk3 = stile([128, 2], "pk3")
            nc.vector.memset(pk3, 0.0)
            nc.scalar.mul(out=pk3[0:32, 0:1], in_=mv3[:, 0:1], mul=float(POS))
            t_g = sc("t_g")
            nc.vector.tensor_mul(t_g[0:32], mv3[:, 0:1], mv3[:, 0:1])
            nc.vector.tensor_add(t_g[0:32], t_g[0:32], mv3[:, 1:2])
            nc.scalar.mul(out=pk3[0:32, 1:2], in_=t_g[0:32], mul=float(POS))
            ps_s3 = ps.tile([1, 2], F32, tag="ps", name="ps_s3")
            nc.tensor.matmul(ps_s3, ones, pk3, start=True, stop=True)
            row3 = stile([1, 8], "row3")
            nc.vector.memset(row3, 0.0)
            nc.vector.tensor_copy(out=row3[0:1, 0:2], in_=ps_s3)
            d4i = dram.tile([8], F32, name="d4i")
            d4o = dram.tile([8], F32, name="d4o")
            nc.sync.dma_start(out=d4i, in_=row3)
            nc.gpsimd.collective_compute(
                "AllReduce", mybir.AluOpType.add,
                replica_groups=[list(range(N_CORES))],
                ins=[d4i.opt()], outs=[d4o.opt()])
            g4 = stile([128, 8], "g4")
            nc.sync.dma_start(out=g4, in_=bass.AP(
                tensor=d4o.tensor, offset=d4o.offset, ap=[[0, 128]] + list(d4o.ap)))

            mu3, r3 = gn_mu_r(g4, 0, 1, N3, "3")
            al3 = stile([128, 1], "al3")
            nc.vector.tensor_mul(al3, r3, _col(pp, 6))
            be3 = stile([128, 1], "be3")
            nc.vector.tensor_mul(be3, mu3, al3)
            nc.vector.tensor_sub(be3, _col(pp, 7), be3)

            # final affine split across two engines, with the second-half DMA
            # able to start as soon as its half is done
            half = POS // 2
            nc.vector.tensor_scalar(out=y3[:, 0:half], in0=y3[:, 0:half],
                                    scalar1=al3[0:32], scalar2=be3[0:32],
                                    op0=mybir.AluOpType.mult,
                                    op1=mybir.AluOpType.add)
            nc.gpsimd.tensor_scalar(out=y3[:, half:POS], in0=y3[:, half:POS],
                                    scalar1=al3[0:32], scalar2=be3[0:32],
                                    op0=mybir.AluOpType.mult,
                                    op1=mybir.AluOpType.add)
            nc.sync.dma_start(out=out_d[:, 0:half], in_=y3[:, 0:half])
            nc.sync.dma_start(out=out_d[:, half:POS], in_=y3[:, half:POS])

    nc.compile()
    return nc


def _host_prep(inputs):
    x = np.asarray(inputs['x'], np.float32).reshape(CIN, S, S, S, S)
    g0w = np.asarray(inputs['g0_w'], np.float32)
    g0b = np.asarray(inputs['g0_b'], np.float32)
    W1 = np.asarray(inputs['w1'], np.float32).reshape(HID, CIN)
    gn1w = np.asarray(inputs['gn1_w'], np.float32)
    gn1b = np.asarray(inputs['gn1_b'], np.float32)
    w2 = np.asarray(inputs['w2'], np.float32).reshape(HID, HID, 3, 3, 3, 3)
    gn2w = np.asarray(inputs['gn2_w'], np.float32)
    gn2b = np.asarray(inputs['gn2_b'], np.float32)
    se1 = np.asarray(inputs['se_w1'], np.float32)   # [8,128]
    se2 = np.asarray(inputs['se_w2'], np.float32)   # [128,8]
    W3 = np.asarray(inputs['w3'], np.float32).reshape(CIN, HID)
    gn3w = np.asarray(inputs['gn3_w'], np.float32)
    gn3b = np.asarray(inputs['gn3_b'], np.float32)

    w1fold = W1 * g0w[None, :]
    w1rep = np.zeros((128, 128), np.float32)
    for j in range(4):
        w1rep[32 * j:32 * j + 32, :] = w1fold.T
    u = W1 @ g0b
    v = W1 @ g0w
    w2t = np.ascontiguousarray(
        w2.transpose(1, 2, 3, 4, 5, 0).reshape(HID, 81 * HID)).astype(
            ml_dtypes.bfloat16)

    params = np.zeros((128, 192), np.float32)
    params[:, 0] = u
    params[:, 1] = v
    params[:, 2] = gn1w
    params[:, 3] = gn1b
    params[:, 4] = gn2w
    params[:, 5] = gn2b
    params[0:32, 6] = gn3w
    params[0:32, 7] = gn3b
    params[:, 10] = u.sum()
    params[:, 11] = v.sum()
    params[:, 12] = (u * u).sum()
    params[:, 13] = (u * v).sum()
    params[:, 14] = (v * v).sum()
    params[:, 16:24] = se1.T
    params[:, 24:56] = W3.T
    params[0:8, 56:184] = se2.T

    xp = np.zeros((CIN, S + 2, S, S, S), np.float32)
    xp[:, 1:S + 1] = x

    in_maps = []
    for k in range(N_CORES):
        p = params.copy()
        p[:, 8] = 0.0 if k == 0 else 1.0
        p[:, 9] = 0.0 if k == N_CORES - 1 else 1.0
        # stored plane order: [owned0, owned1, haloL, haloR]
        idx = [2 * k + 1, 2 * k + 2, 2 * k, 2 * k + 3]
        shard = np.ascontiguousarray(
            xp[:, idx].transpose(1, 0, 2, 3, 4).reshape(128, PLANE))
        in_maps.append({"xs": shard, "w1rep": w1rep, "w2t": w2t, "params": p})
    return in_maps


def kernel(**inputs):
    if "nc" not in _cache:
        _cache["nc"] = build_program()
    nc = _cache["nc"]
    in_maps = _host_prep(inputs)
    res = run_bass_kernel_spmd(nc, in_maps, core_ids=list(range(N_CORES)))
    out = np.empty((1, CIN, S, S, S, S), np.float32)
    for k in range(N_CORES):
        out[0, :, 2 * k:2 * k + 2] = res.results[k]["out"].reshape(CIN, 2, S, S, S)
    return out


def run_traced(inputs):
    """Like kernel() but with NTFF tracing; returns (out, BassKernelResults)."""
    if "nc" not in _cache:
        _cache["nc"] = build_program()
    nc = _cache["nc"]
    in_maps = _host_prep(inputs)
    res = run_bass_kernel_spmd(nc, in_maps, core_ids=list(range(N_CORES)),
                               trace=True)
    out = np.empty((1, CIN, S, S, S, S), np.float32)
    for k in range(N_CORES):
        out[0, :, 2 * k:2 * k + 2] = res.results[k]["out"].reshape(CIN, 2, S, S, S)
    return out, res


# revision 42
# speedup vs baseline: 1.0061x; 1.0061x over previous
"""MBConv (4D spatial, 16^4) on 8 TRN2 NeuronCores.

Sharding: spatial-parallel over the first spatial dim X (16 planes ->
2 owned planes per core + 1 halo plane each side, shipped from host).

Math (all on device except weight-only constant folding on host):
  GN0+conv1+GN1 folded: A' = (W1 * g0_w) . x computed once; the two
  global groupnorms reduce to 6 scalars in ONE AllReduce:
    [Sum(A'), Sum(A'^2), Sum(u*SA), Sum(v*SA), Sum(x), Sum(x^2)]
  with u = W1.g0_b, v = W1.g0_w (host constants); then
  h1 = gelu(alpha1 * A' + beta1) per hidden channel.
  conv2 = 81 accumulating PE matmuls per PSUM bank over a zero-padded
  [128ch, 4planes, 18,18,18] SBUF tile (float32r fast-fp32 mode).
  GN2 -> AllReduce(2 scalars); gelu fused with SE partial-mean accum.
  SE mean -> AllReduce(128); SE MLP on-device; scale folded into w3.
  conv3; GN3 -> AllReduce(2 scalars); affine; DMA out.
"""

import sys
sys.path.insert(0, '/opt/trn_rl_repo')

import numpy as np
import ml_dtypes

import concourse.bass as bass
import concourse.bacc as bacc
import concourse.tile as tile
import concourse.mybir as mybir
from concourse.bass_utils import run_bass_kernel_spmd

F32 = mybir.dt.float32
F32R = mybir.dt.float32r
BF16 = mybir.dt.bfloat16
AF = mybir.ActivationFunctionType

N_CORES = 8
S = 16
CIN = 32
HID = 128
EPS = 1e-5
PLANE = S * S * S            # 4096 positions per x-plane
PPAD = 18 * 18 * 18          # padded plane (z/y/w pad 1)
NPL = 4                      # stored planes per core (2 owned + 2 halo)
POS = 2 * PLANE              # owned positions per core
P_SP = S ** 4                # 65536 global spatial positions
NX = CIN * P_SP
N1 = HID * P_SP
N3 = CIN * P_SP

_cache = {}


def _col(t, i):
    return t[:, i:i + 1]


def build_program(trace_scopes=False):
    nc = bacc.Bacc("TRN2", target_bir_lowering=False, debug=False,
                   enable_asserts=False, num_devices=N_CORES)

    xs_d = nc.dram_tensor("xs", [128, PLANE], F32R, kind="ExternalInput").ap()
    w1_d = nc.dram_tensor("w1rep", [128, 128], F32R, kind="ExternalInput").ap()
    w2_d = nc.dram_tensor("w2t", [128, 81 * 128], BF16, kind="ExternalInput").ap()
    pp_d = nc.dram_tensor("params", [128, 192], F32, kind="ExternalInput").ap()
    out_d = nc.dram_tensor("out", [CIN, POS], F32, kind="ExternalOutput").ap()

    with tile.TileContext(nc) as tc:
        with tc.tile_pool(name="big", bufs=1) as big, \
             tc.tile_pool(name="small", bufs=1) as small, \
             tc.tile_pool(name="scr", bufs=24) as scr, \
             tc.tile_pool(name="ps", bufs=8, space="PSUM") as ps, \
             tc.tile_pool(name="dram", bufs=1, space="DRAM") as dram:

            def stile(shape, name, pool=None):
                return (pool or small).tile(shape, F32, name=name)

            def sc(name):
                return scr.tile([128, 1], F32, tag="scr", name=name)

            # ---- persistent SBUF tensors ----
            x_sb = big.tile([128, PLANE], F32R, name="x_sb")
            w1_sb = big.tile([128, 128], F32R, name="w1_sb")
            w2_sb = big.tile([128, 81 * 128], BF16, name="w2_sb")
            pp = big.tile([128, 192], F32, name="pp")
            h1 = big.tile([128, NPL * PPAD], BF16, name="h1", tag="bigslot")
            h2 = big.tile([128, 2 * PLANE], F32R, name="h2")

            nc.sync.dma_start(out=x_sb, in_=xs_d)
            nc.sync.dma_start(out=pp, in_=pp_d)
            nc.sync.dma_start(out=w1_sb, in_=w1_d)
            nc.sync.dma_start(out=w2_sb, in_=w2_d)

            h1f5 = h1.rearrange("p (j y z w) -> p j y z w", j=NPL, y=18, z=18, w=18)
            h1pl = h1.rearrange("p (j r) -> p j r", j=NPL, r=PPAD)
            # zero h1 (padding must be 0)
            for j in range(NPL):
                eng = nc.vector if j % 2 == 0 else nc.gpsimd
                eng.memset(h1pl[:, j, :], 0.0)

            def interior(j):
                return h1f5[:, j, 1:17, 1:17, 1:17]

            def interior_chunk(j, n):  # output y-pair chunk [128,(2,16,16)]
                return h1f5[:, j, 1 + 2 * n:3 + 2 * n, 1:17, 1:17]

            eps_t = stile([128, 1], "eps_t")
            nc.vector.memset(eps_t, EPS)
            ones = stile([128, 1], "ones")
            nc.vector.memset(ones, 1.0)

            # ---- conv1: A' = (W1*g0w) . x  on all 4 planes ----
            # Shard partition packing puts OWNED planes on partitions 0:64
            # (stored order [owned0, owned1, haloL, haloR]); LOC maps stored
            # plane index -> local x position in the padded h1 buffer.
            # A'-stats (owned planes only) taken from the contiguous PSUM
            # tiles before eviction (bn_stats reduces innermost dim only).
            LOC = (1, 2, 0, 3)
            # stage A' contiguously; h1 keeps few writers (memset+gelu+mask)
            # so conv2's dependency tracking stays cheap
            aprime = big.tile([128, NPL * PLANE], BF16, name="aprime")
            ap5 = aprime.rearrange("p (s y z w) -> p s y z w",
                                   s=NPL, y=16, z=16, w=16)
            sta = stile([128, 16, 6], "sta")
            for sj in range(NPL):
                for n in range(8):
                    pt = ps.tile([128, 512], F32, tag="ps", name=f"c1_{sj}_{n}")
                    nc.tensor.matmul(
                        pt,
                        w1_sb[32 * sj:32 * sj + 32, :],
                        x_sb[32 * sj:32 * sj + 32, bass.ts(n, 512)],
                        start=True, stop=True, tile_position=(32 * sj, 0))
                    nc.scalar.copy(
                        out=aprime[:, bass.ts(sj * 8 + n, 512)], in_=pt)
                    if sj < 2:
                        nc.vector.bn_stats(out=sta[:, sj * 8 + n, :], in_=pt)

            # ---- stats for folded GN0+GN1 (owned data only) ----
            stx = stile([128, 8, 6], "stx")
            x_f32 = x_sb.bitcast(F32)
            for c in range(8):
                nc.vector.bn_stats(out=stx[0:64, c, :],
                                   in_=x_f32[0:64, bass.ts(c, 512)])
            mvx = stile([128, 2], "mvx")
            nc.vector.bn_aggr(out=mvx[0:64, :], in_=stx[0:64])

            mva = stile([128, 2], "mva")
            nc.vector.bn_aggr(out=mva, in_=sta)

            pk = stile([128, 6], "pk")
            nc.vector.memset(pk, 0.0)
            # col0: SA_o = mean*POS ; col1: SAA_o = (var+mean^2)*POS
            nc.scalar.mul(out=_col(pk, 0), in_=_col(mva, 0), mul=float(POS))
            t_a = sc("t_a")
            nc.vector.tensor_mul(t_a, _col(mva, 0), _col(mva, 0))
            nc.vector.tensor_add(t_a, t_a, _col(mva, 1))
            nc.scalar.mul(out=_col(pk, 1), in_=t_a, mul=float(POS))
            nc.vector.tensor_mul(_col(pk, 2), _col(pp, 0), _col(pk, 0))   # u*SA
            nc.vector.tensor_mul(_col(pk, 3), _col(pp, 1), _col(pk, 0))   # v*SA
            # x stats on owned planes (partitions 0:64, 4096 positions each)
            nc.scalar.mul(out=pk[0:64, 4:5], in_=mvx[0:64, 0:1], mul=float(PLANE))
            t_b = sc("t_b")
            nc.vector.tensor_mul(t_b[0:64], mvx[0:64, 0:1], mvx[0:64, 0:1])
            nc.vector.tensor_add(t_b[0:64], t_b[0:64], mvx[0:64, 1:2])
            nc.scalar.mul(out=pk[0:64, 5:6], in_=t_b[0:64], mul=float(PLANE))

            ps_s1 = ps.tile([1, 6], F32, tag="ps", name="ps_s1")
            nc.tensor.matmul(ps_s1, ones, pk, start=True, stop=True)

            row1 = stile([1, 8], "row1")
            nc.vector.memset(row1, 0.0)
            nc.vector.tensor_copy(out=row1[0:1, 0:6], in_=ps_s1)
            d1i = dram.tile([8], F32, name="d1i")
            d1o = dram.tile([8], F32, name="d1o")
            nc.sync.dma_start(out=d1i, in_=row1)
            nc.gpsimd.collective_compute(
                "AllReduce", mybir.AluOpType.add,
                replica_groups=[list(range(N_CORES))],
                ins=[d1i.opt()], outs=[d1o.opt()])
            g1 = stile([128, 8], "g1")
            nc.sync.dma_start(out=g1, in_=bass.AP(
                tensor=d1o.tensor, offset=d1o.offset, ap=[[0, 128]] + list(d1o.ap)))

            # ---- scalar chain (replicated on 128 partitions) ----
            def gn_mu_r(g, i_sum, i_ss, nval, tag):
                mu = stile([128, 1], f"mu_{tag}")
                nc.scalar.mul(out=mu, in_=_col(g, i_sum), mul=1.0 / nval)
                ex2 = sc(f"ex2_{tag}")
                nc.scalar.mul(out=ex2, in_=_col(g, i_ss), mul=1.0 / nval)
                var = sc(f"var_{tag}")
                nc.vector.tensor_mul(var, mu, mu)
                nc.vector.tensor_sub(var, ex2, var)
                std = sc(f"std_{tag}")
                nc.scalar.activation(out=std, in_=var, func=AF.Sqrt, bias=eps_t)
                r = stile([128, 1], f"r_{tag}")
                nc.vector.reciprocal(r, std)
                return mu, r

            # g1 cols: 0 SumSA, 1 SAA, 2 SumU.SA, 3 SumV.SA, 4 Sx, 5 Sxx
            mu0, r0 = gn_mu_r(g1, 4, 5, NX, "0")
            q = stile([128, 1], "q")
            nc.vector.tensor_mul(q, mu0, r0)
            scsa = sc("scsa")                       # Sum(c*SA) = col2 - q*col3
            nc.vector.tensor_mul(scsa, q, _col(g1, 3))
            nc.vector.tensor_sub(scsa, _col(g1, 2), scsa)
            s_c = sc("s_c")                         # Sum(c) = Su - q*Sv
            nc.vector.tensor_mul(s_c, q, _col(pp, 11))
            nc.vector.tensor_sub(s_c, _col(pp, 10), s_c)
            scc = sc("scc")                         # Sum(c^2)
            t_c = sc("t_c")
            nc.vector.tensor_mul(t_c, q, _col(pp, 13))
            nc.scalar.mul(out=t_c, in_=t_c, mul=2.0)
            nc.vector.tensor_sub(scc, _col(pp, 12), t_c)
            nc.vector.tensor_mul(t_c, q, q)
            nc.vector.tensor_mul(t_c, t_c, _col(pp, 14))
            nc.vector.tensor_add(scc, scc, t_c)
            # mu1
            mu1 = stile([128, 1], "mu1")
            nc.vector.tensor_mul(mu1, r0, _col(g1, 0))
            t_d = sc("t_d")
            nc.scalar.mul(out=t_d, in_=s_c, mul=float(P_SP))
            nc.vector.tensor_add(mu1, mu1, t_d)
            nc.scalar.mul(out=mu1, in_=mu1, mul=1.0 / N1)
            # var1 = (r0^2*SAA + 2 r0 scsa + P*scc)/N1 - mu1^2
            v1 = sc("v1")
            nc.vector.tensor_mul(v1, r0, r0)
            nc.vector.tensor_mul(v1, v1, _col(g1, 1))
            t_e = sc("t_e")
            nc.vector.tensor_mul(t_e, r0, scsa)
            nc.scalar.mul(out=t_e, in_=t_e, mul=2.0)
            nc.vector.tensor_add(v1, v1, t_e)
            nc.scalar.mul(out=t_e, in_=scc, mul=float(P_SP))
            nc.vector.tensor_add(v1, v1, t_e)
            nc.scalar.mul(out=v1, in_=v1, mul=1.0 / N1)
            nc.vector.tensor_mul(t_e, mu1, mu1)
            nc.vector.tensor_sub(v1, v1, t_e)
            std1 = sc("std1")
            nc.scalar.activation(out=std1, in_=v1, func=AF.Sqrt, bias=eps_t)
            r1 = stile([128, 1], "r1")
            nc.vector.reciprocal(r1, std1)
            al1 = stile([128, 1], "al1")
            nc.vector.tensor_mul(al1, r0, r1)
            nc.vector.tensor_mul(al1, al1, _col(pp, 2))
            be1 = stile([128, 1], "be1")
            nc.vector.tensor_mul(be1, q, _col(pp, 1))        # q*v
            nc.vector.tensor_sub(be1, _col(pp, 0), be1)      # c = u - q*v
            nc.vector.tensor_sub(be1, be1, mu1)              # c - mu1
            nc.vector.tensor_mul(be1, be1, r1)
            nc.vector.tensor_mul(be1, be1, _col(pp, 2))
            nc.vector.tensor_add(be1, be1, _col(pp, 3))

            # ---- h1 = gelu(alpha1*A' + beta1); mask edge halos ----
            # order: haloL first+mask, then owned planes, then haloR — conv2's
            # first output plane needs local planes 0..2; plane 3 gelu
            # overlaps conv2's start
            for sj in (2, 0, 1, 3):
                lj = LOC[sj]
                nc.scalar.activation(out=interior(lj), in_=ap5[:, sj],
                                     func=AF.Gelu, bias=be1, scale=al1)
                if lj == 0:
                    nc.vector.tensor_scalar_mul(out=interior(0), in0=interior(0),
                                                scalar1=_col(pp, 8))
                elif lj == NPL - 1:
                    nc.vector.tensor_scalar_mul(out=interior(NPL - 1),
                                                in0=interior(NPL - 1),
                                                scalar1=_col(pp, 9))

            # ---- conv2: 3^4, 81 taps, accumulate in PSUM ----
            h1r5 = h1f5
            w2r = w2_sb
            sth = stile([128, 16, 6], "sth")
            for j in range(2):
                for b in range(8):
                    # all 81 taps accumulate into ONE psum bank back-to-back
                    # (bank-contiguous: avoids PSUM queue cycling + keeps the
                    # LDWEIGHTS/MATMUL pipeline warm); eviction of bank b
                    # overlaps bank b+1's accumulation
                    pt = ps.tile([128, 512], F32, tag="ps", name=f"c2_{j}_{b}")
                    t = 0
                    for dx in range(3):
                        for dy in range(3):
                            for dz in range(3):
                                for dw in range(3):
                                    mov = h1r5[:, j + dx,
                                               2 * b + dy:2 * b + dy + 2,
                                               dz:dz + 16, dw:dw + 16]
                                    nc.tensor.matmul(pt, w2r[:, bass.ts(t, 128)],
                                                     mov,
                                                     start=(t == 0), stop=(t == 80))
                                    t += 1
                    blk = bass.ts(j * 8 + b, 512)
                    nc.scalar.copy(out=h2[:, blk], in_=pt)
                    nc.vector.bn_stats(out=sth[:, j * 8 + b, :],
                                       in_=h2.bitcast(F32)[:, blk])

            mvh = stile([128, 2], "mvh")
            nc.vector.bn_aggr(out=mvh, in_=sth)
            pk2 = stile([128, 2], "pk2")
            nc.scalar.mul(out=_col(pk2, 0), in_=_col(mvh, 0), mul=float(POS))
            t_f = sc("t_f")
            nc.vector.tensor_mul(t_f, _col(mvh, 0), _col(mvh, 0))
            nc.vector.tensor_add(t_f, t_f, _col(mvh, 1))
            nc.scalar.mul(out=_col(pk2, 1), in_=t_f, mul=float(POS))
            ps_s2 = ps.tile([1, 2], F32, tag="ps", name="ps_s2")
            nc.tensor.matmul(ps_s2, ones, pk2, start=True, stop=True)
            row2 = stile([1, 8], "row2")
            nc.vector.memset(row2, 0.0)
            nc.vector.tensor_copy(out=row2[0:1, 0:2], in_=ps_s2)
            d2i = dram.tile([8], F32, name="d2i")
            d2o = dram.tile([8], F32, name="d2o")
            nc.sync.dma_start(out=d2i, in_=row2)
            nc.gpsimd.collective_compute(
                "AllReduce", mybir.AluOpType.add,
                replica_groups=[list(range(N_CORES))],
                ins=[d2i.opt()], outs=[d2o.opt()])
            g2 = stile([128, 8], "g2")
            nc.sync.dma_start(out=g2, in_=bass.AP(
                tensor=d2o.tensor, offset=d2o.offset, ap=[[0, 128]] + list(d2o.ap)))

            mu2, r2 = gn_mu_r(g2, 0, 1, N1, "2")
            al2 = stile([128, 1], "al2")
            nc.vector.tensor_mul(al2, r2, _col(pp, 4))
            be2 = stile([128, 1], "be2")
            nc.vector.tensor_mul(be2, mu2, al2)
            nc.vector.tensor_sub(be2, _col(pp, 5), be2)

            # ---- gelu(GN2) in place + SE partial sums via accum_out ----
            mcols = stile([128, 16], "mcols")
            h2f = h2.bitcast(F32)
            for n in range(16):
                nc.scalar.activation(out=h2[:, bass.ts(n, 512)],
                                     in_=h2f[:, bass.ts(n, 512)],
                                     func=AF.Gelu, bias=be2, scale=al2,
                                     accum_out=mcols[:, n:n + 1])
            m_col = stile([128, 1], "m_col")
            nc.vector.reduce_sum(out=m_col, in_=mcols, axis=mybir.AxisListType.X)
            d3i = dram.tile([128], F32, name="d3i")
            d3o = dram.tile([128], F32, name="d3o")
            nc.sync.dma_start(out=d3i, in_=m_col)
            nc.gpsimd.collective_compute(
                "AllReduce", mybir.AluOpType.add,
                replica_groups=[list(range(N_CORES))],
                ins=[d3i.opt()], outs=[d3o.opt()])
            m_sb = stile([128, 1], "m_sb")
            nc.sync.dma_start(out=m_sb, in_=d3o)

            # ---- SE MLP (tiny, replicated on every core) ----
            m_mean = stile([128, 1], "m_mean")
            nc.scalar.mul(out=m_mean, in_=m_sb, mul=1.0 / P_SP)
            ps_se1 = ps.tile([8, 1], F32, tag="ps", name="ps_se1")
            nc.tensor.matmul(ps_se1, pp[:, 16:24], m_mean, start=True, stop=True)
            y1g = stile([8, 1], "y1g")
            nc.scalar.activation(out=y1g, in_=ps_se1, func=AF.Gelu)
            ps_se2 = ps.tile([128, 1], F32, tag="ps", name="ps_se2")
            nc.tensor.matmul(ps_se2, pp[0:8, 56:184], y1g, start=True, stop=True)
            s_sb = stile([128, 1], "s_sb")
            nc.scalar.activation(out=s_sb, in_=ps_se2, func=AF.Sigmoid)
            w3s = small.tile([128, 32], F32R, name="w3s")
            nc.vector.tensor_scalar_mul(out=w3s, in0=pp[:, 24:56], scalar1=s_sb)

            # ---- conv3 (+ stats), y3 shares the h1 slot ----
            y3 = big.tile([CIN, POS], F32, name="y3", tag="bigslot")
            st3 = stile([32, 16, 6], "st3")
            for n in range(16):
                pt3 = ps.tile([32, 512], F32, tag="ps", name=f"c3_{n}")
                nc.tensor.matmul(pt3, w3s, h2[:, bass.ts(n, 512)],
                                 start=True, stop=True)
                nc.scalar.copy(out=y3[:, bass.ts(n, 512)], in_=pt3)
                nc.vector.bn_stats(out=st3[:, n, :], in_=pt3)
            mv3 = stile([32, 2], "mv3")
            nc.vector.bn_aggr(out=mv3, in_=st3)
            pk3 = stile([128, 2], "pk3")
            nc.vector.memset(pk3, 0.0)
            nc.scalar.mul(out=pk3[0:32, 0:1], in_=mv3[:, 0:1], mul=float(POS))
            t_g = sc("t_g")
            nc.vector.tensor_mul(t_g[0:32], mv3[:, 0:1], mv3[:, 0:1])
            nc.vector.tensor_add(t_g[0:32], t_g[0:32], mv3[:, 1:2])
            nc.scalar.mul(out=pk3[0:32, 1:2], in_=t_g[0:32], mul=float(POS))
            ps_s3 = ps.tile([1, 2], F32, tag="ps", name="ps_s3")
            nc.tensor.matmul(ps_s3, ones, pk3, start=True, stop=True)
            row3 = stile([1, 8], "row3")
            nc.vector.memset(row3, 0.0)
            nc.vector.tensor_copy(out=row3[0:1, 0:2], in_=ps_s3)
            d4i = dram.tile([8], F32, name="d4i")
            d4o = dram.tile([8], F32, name="d4o")
            nc.sync.dma_start(out=d4i, in_=row3)
            nc.gpsimd.collective_compute(
                "AllReduce", mybir.AluOpType.add,
                replica_groups=[list(range(N_CORES))],
                ins=[d4i.opt()], outs=[d4o.opt()])
            g4 = stile([128, 8], "g4")
            nc.sync.dma_start(out=g4, in_=bass.AP(
                tensor=d4o.tensor, offset=d4o.offset, ap=[[0, 128]] + list(d4o.ap)))

            mu3, r3 = gn_mu_r(g4, 0, 1, N3, "3")
            al3 = stile([128, 1], "al3")
            nc.vector.tensor_mul(al3, r3, _col(pp, 6))
            be3 = stile([128, 1], "be3")
            nc.vector.tensor_mul(be3, mu3, al3)
            nc.vector.tensor_sub(be3, _col(pp, 7), be3)

            # final affine split across two engines, with the second-half DMA
            # able to start as soon as its half is done
            half = POS // 2
            nc.vector.tensor_scalar(out=y3[:, 0:half], in0=y3[:, 0:half],
                                    scalar1=al3[0:32], scalar2=be3[0:32],
                                    op0=mybir.AluOpType.mult,
                                    op1=mybir.AluOpType.add)
            nc.gpsimd.tensor_scalar(out=y3[:, half:POS], in0=y3[:, half:POS],
                                    scalar1=al3[0:32], scalar2=be3[0:32],
                                    op0=mybir.AluOpType.mult,
                                    op1=mybir.AluOpType.add)
            nc.sync.dma_start(out=out_d[:, 0:half], in_=y3[:, 0:half])
            nc.sync.dma_start(out=out_d[:, half:POS], in_=y3[:, half:POS])

    nc.compile()
    return nc


def _host_prep(inputs):
    x = np.asarray(inputs['x'], np.float32).reshape(CIN, S, S, S, S)
    g0w = np.asarray(inputs['g0_w'], np.float32)
    g0b = np.asarray(inputs['g0_b'], np.float32)
    W1 = np.asarray(inputs['w1'], np.float32).reshape(HID, CIN)
    gn1w = np.asarray(inputs['gn1_w'], np.float32)
    gn1b = np.asarray(inputs['gn1_b'], np.float32)
    w2 = np.asarray(inputs['w2'], np.float32).reshape(HID, HID, 3, 3, 3, 3)
    gn2w = np.asarray(inputs['gn2_w'], np.float32)
    gn2b = np.asarray(inputs['gn2_b'], np.float32)
    se1 = np.asarray(inputs['se_w1'], np.float32)   # [8,128]
    se2 = np.asarray(inputs['se_w2'], np.float32)   # [128,8]
    W3 = np.asarray(inputs['w3'], np.float32).reshape(CIN, HID)
    gn3w = np.asarray(inputs['gn3_w'], np.float32)
    gn3b = np.asarray(inputs['gn3_b'], np.float32)

    w1fold = W1 * g0w[None, :]
    w1rep = np.zeros((128, 128), np.float32)
    for j in range(4):
        w1rep[32 * j:32 * j + 32, :] = w1fold.T
    u = W1 @ g0b
    v = W1 @ g0w
    w2t = np.ascontiguousarray(
        w2.transpose(1, 2, 3, 4, 5, 0).reshape(HID, 81 * HID)).astype(
            ml_dtypes.bfloat16)

    params = np.zeros((128, 192), np.float32)
    params[:, 0] = u
    params[:, 1] = v
    params[:, 2] = gn1w
    params[:, 3] = gn1b
    params[:, 4] = gn2w
    params[:, 5] = gn2b
    params[0:32, 6] = gn3w
    params[0:32, 7] = gn3b
    params[:, 10] = u.sum()
    params[:, 11] = v.sum()
    params[:, 12] = (u * u).sum()
    params[:, 13] = (u * v).sum()
    params[:, 14] = (v * v).sum()
    params[:, 16:24] = se1.T
    params[:, 24:56] = W3.T
    params[0:8, 56:184] = se2.T

    xp = np.zeros((CIN, S + 2, S, S, S), np.float32)
    xp[:, 1:S + 1] = x

    in_maps = []
    for k in range(N_CORES):
        p = params.copy()
        p[:, 8] = 0.0 if k == 0 else 1.0
        p[:, 9] = 0.0 if k == N_CORES - 1 else 1.0
        # stored plane order: [owned0, owned1, haloL, haloR]
        idx = [2 * k + 1, 2 * k + 2, 2 * k, 2 * k + 3]
        shard = np.ascontiguousarray(
            xp[:, idx].transpose(1, 0, 2, 3, 4).reshape(128, PLANE))
        in_maps.append({"xs": shard, "w1rep": w1rep, "w2t": w2t, "params": p})
    return in_maps


def kernel(**inputs):
    if "nc" not in _cache:
        _cache["nc"] = build_program()
    nc = _cache["nc"]
    in_maps = _host_prep(inputs)
    res = run_bass_kernel_spmd(nc, in_maps, core_ids=list(range(N_CORES)))
    out = np.empty((1, CIN, S, S, S, S), np.float32)
    for k in range(N_CORES):
        out[0, :, 2 * k:2 * k + 2] = res.results[k]["out"].reshape(CIN, 2, S, S, S)
    return out


def run_traced(inputs):
    """Like kernel() but with NTFF tracing; returns (out, BassKernelResults)."""
    if "nc" not in _cache:
        _cache["nc"] = build_program()
    nc = _cache["nc"]
    in_maps = _host_prep(inputs)
    res = run_bass_kernel_spmd(nc, in_maps, core_ids=list(range(N_CORES)),
                               trace=True)
    out = np.empty((1, CIN, S, S, S, S), np.float32)
    for k in range(N_CORES):
        out[0, :, 2 * k:2 * k + 2] = res.results[k]["out"].reshape(CIN, 2, S, S, S)
    return out, res
